# revision 1
# baseline (speedup 1.0000x reference)
"""Trainium2 Bass kernel v2 for nn_MoDBlock (mixture-of-depths block).

Per batch sequence b:
  scores = x_b @ w_router, computed exactly as (xf8 + r) @ w_router where
           xf8 = fp8(x) and r = bf16(x - xf8); the split halves the score
           DMA bytes while keeping ~1e-4 relative accuracy, far below the
           ~8e-4 top-k boundary gap, so the selected set matches f32 topk
  pos    = top-512 token positions (exact kth_largest threshold + gpsimd
           sparse_gather compaction), ascending
  tokens = x_b[pos] as fp8, gathered TRANSPOSED by gpsimd dma_gather
  causal 16-head attention over the 512 compacted tokens + w_proj
  layernorm + MLP (gelu-tanh)
  host: out[b, pos] += (partial_even + partial_odd) / 64

Sharding: 8 cores, core c = (batch b=c//2, hidden-half e=c%2). Scoring is
split across the pair (each core scores its half; an 8KB AllGather merges
— the only collective). Selection / gather / qkv / attention / proj / LN
are computed redundantly by both cores of a pair; the MLP is split by
hidden columns (w_fc cols / w_out rows) through the *input weight data*,
so the compiled program is identical on every core. Each core returns its
partial out-projection [512,1024] bf16 (x64 via the host-side weight
pre-scale); the host sums the pair, divides by 64, and scatter-adds into
x (the f32 residual stays exact on host).

Precision: weights are pre-scaled x64 and cast to fp8e4 (DoubleRow
matmuls: 2x PE throughput, half the weight DMA bytes); activations flow
bf16/fp8 with all matmul accumulation in f32 PSUM.

Attention layout: S^T[k,q] = (kT_h)^T @ qT_h per 128-key block, exp'd
directly into SBUF (bf16) so PV needs no P transposes; causal masking is
a tril multiply on the diagonal block; softmax row sums come from a
ones-column matmul alongside PV, and normalization is a per-partition
scalar multiply on the row-major PV output.

DMA scheduling: the wire is FIFO, so the big weight loads are gated with
tiny dummy data-deps (wqkv on the merged scores, the rest on the gathered
tokens) to keep them out of the latency-critical selection path.
"""

import sys
from contextlib import ExitStack

sys.path.insert(0, "/opt/trn_rl_repo")

import numpy as np
import ml_dtypes

from concourse import bass, mybir, tile, bacc
from concourse.bass_utils import run_bass_kernel_spmd

BF16NP = ml_dtypes.bfloat16
F8NP = ml_dtypes.float8_e4m3
F32 = mybir.dt.float32
BF = mybir.dt.bfloat16
F8 = mybir.dt.float8e4
I32 = mybir.dt.int32
I16 = mybir.dt.int16
U32 = mybir.dt.uint32
AF = mybir.ActivationFunctionType
OP = mybir.AluOpType
DR = mybir.MatmulPerfMode.DoubleRow

D = 1024
S = 4096
B = 4
H = 16
HD = 64
K = 512
FCH = 2048           # fc hidden columns per core (4096 / 2)
WS = 64.0            # weight pre-scale folded into fp8 weights
WSI = 1.0 / WS


def build_program(n_cores=8, gelu_exact=False, collectives=True, debug=False):
    nc = bacc.Bacc(
        "TRN2", target_bir_lowering=False, debug=False, num_devices=n_cores
    )

    # ---- I/O ----
    xf8 = nc.dram_tensor("xf8", [S, D], F8, kind="ExternalInput")
    xrp = nc.dram_tensor("xrp", [S // 2, D], BF, kind="ExternalInput")
    gidx = nc.dram_tensor("gidx", [128, 128], I16, kind="ExternalInput")
    w8rep = nc.dram_tensor("w8rep", [128, 4, 2, 128], F8, kind="ExternalInput")
    wqkv = nc.dram_tensor("wqkv", [128, 4, 2, 3 * D], F8, kind="ExternalInput")
    wproj = nc.dram_tensor("wproj", [128, 4, 2, D], F8, kind="ExternalInput")
    wfc = nc.dram_tensor("wfc", [128, 4, 2, FCH], F8, kind="ExternalInput")
    wout = nc.dram_tensor("wout", [128, 8, 2, D], F8, kind="ExternalInput")
    wrr = nc.dram_tensor("wrouter_rep", [128, D], F32, kind="ExternalInput")
    identbd = nc.dram_tensor("identb", [128, 128], BF, kind="ExternalInput")
    ident32d = nc.dram_tensor("ident32", [128, 128], F32, kind="ExternalInput")
    ones32d = nc.dram_tensor("ones32", [1, 16], F32, kind="ExternalInput")
    iota16d = nc.dram_tensor("iota16", [16, 256], F32, kind="ExternalInput")
    rep16d = nc.dram_tensor("rep16", [16, 128], F32, kind="ExternalInput")
    onesbd = nc.dram_tensor("onesb", [128, 8], BF, kind="ExternalInput")
    trilqd = nc.dram_tensor("trilq", [128, 128], BF, kind="ExternalInput")

    updp = nc.dram_tensor("updp", [128, 4, D], BF, kind="ExternalOutput")
    pos_out = nc.dram_tensor("pos_out", [16, 32], I32, kind="ExternalOutput")
    nf_out = nc.dram_tensor("nf_out", [1, 1], U32, kind="ExternalOutput")
    groups = [[i, i + 1] for i in range(0, n_cores, 2)]
    ag_out = nc.dram_tensor("ag_out", [256, 16], F32)
    ag_out2 = nc.dram_tensor("ag_out2", [2, S // 2], F32)
    if debug:
        tokT_dbg = nc.dram_tensor("tokT_dbg", [128, 8, K], F8,
                                  kind="ExternalOutput")
        attn_dbg = nc.dram_tensor("attn_dbg", [128, 4, D], BF,
                                  kind="ExternalOutput")
        sc_dbg = nc.dram_tensor("sc_dbg", [128, 32], F32,
                                kind="ExternalOutput")
        s16_dbg = nc.dram_tensor("s16_dbg", [16, 256], F32,
                                 kind="ExternalOutput")

    with tile.TileContext(nc) as tc, ExitStack() as ctx:
        const = ctx.enter_context(tc.tile_pool(name="const", bufs=1))
        wp = ctx.enter_context(tc.tile_pool(name="wp", bufs=1))
        xsp = ctx.enter_context(tc.tile_pool(name="xsp", bufs=3))
        act = ctx.enter_context(tc.tile_pool(name="act", bufs=1))
        ptp = ctx.enter_context(tc.tile_pool(name="ptp", bufs=3))
        sml = ctx.enter_context(tc.tile_pool(name="sml", bufs=3))
        mm = ctx.enter_context(tc.tile_pool(name="mm", bufs=3, space="PSUM"))
        ov = ctx.enter_context(tc.tile_pool(name="ov", bufs=2, space="PSUM"))
        ovr = ctx.enter_context(tc.tile_pool(name="ovr", bufs=1, space="PSUM"))
        trp = ctx.enter_context(tc.tile_pool(name="trp", bufs=2, space="PSUM"))
        drp = ctx.enter_context(tc.tile_pool(name="drp", bufs=1, space="DRAM"))

        # ---- phase 1: router scores over this core's half of x ----
        # score*64 = x8*w8 + r'*w64 with x8 = fp8(x), w8 = fp8(64w),
        # r' = bf16(x - x8*(w8/w64)) host-computed: algebraically exact up
        # to the bf16 rounding of the small r' (~1e-4 of score, far below
        # the ~5e-2 scaled top-k boundary gap). Pass A runs on the idle
        # Tensor engine (fp8 DoubleRow, replicated router weights) from
        # transposed gather-streams; pass B is 16 DVE STTs (half the old
        # DVE load). The partials merge after two tiny pair-AllGathers.
        wrr_sb = const.tile([128, D], F32, tag="wrr")
        nc.scalar.dma_start(out=wrr_sb[:], in_=wrr[:, :])
        gix = const.tile([128, 128], I16, tag="gix")
        nc.scalar.dma_start(out=gix[:], in_=gidx[:, :])
        w8_sb = const.tile([128, 4, 2, 128], F8, tag="w8rep")
        nc.scalar.dma_start(out=w8_sb[:], in_=w8rep[:, :, :, :])
        scores = const.tile([128, 32], F32, tag="scores")
        scB = const.tile([128, 32], F32, tag="scB")
        sc_half = const.tile([128, 16], F32, tag="scorehalf")
        scrow = const.tile([1, S // 2], F32, tag="scrow")
        for q in range(4):
            x8T = xsp.tile([128, 8, 512], F8, tag="x8", name=f"x8T{q}")
            nc.gpsimd.dma_gather(
                out_ap=x8T[:, :, :], in_ap=xf8[:, :],
                idxs_ap=gix[:, q * 32:(q + 1) * 32],
                num_idxs=512, num_idxs_reg=512, elem_size=D, transpose=True,
            )
            xr = xsp.tile([128, 4, D], BF, tag="xr", name=f"xr_{q}")
            nc.sync.dma_start(
                out=xr[:],
                in_=xrp[q * 512:(q + 1) * 512, :].rearrange(
                    "(i p) d -> p i d", p=128),
            )
            x8Tg = x8T[:].rearrange("p c t -> p (c t)").rearrange(
                "p (g t j) -> p g j t", g=4, j=2)
            sq = ov.tile([128, 512], F32, tag="ov", name=f"sq{q}")
            for g in range(4):
                nc.tensor.matmul(
                    out=sq[:], lhsT=w8_sb[:, g, :, :], rhs=x8Tg[:, g, :, :],
                    start=(g == 0), stop=(g == 3), perf_mode=DR,
                )
            nc.scalar.activation(out=scrow[0:1, q * 512:(q + 1) * 512],
                                 in_=sq[0:1, :], func=AF.Copy)
            for i in range(4):
                t = q * 4 + i
                nc.vector.scalar_tensor_tensor(
                    out=xr[:, i, :], in0=xr[:, i, :], scalar=0.0,
                    in1=wrr_sb[:], op0=OP.add, op1=OP.mult,
                    accum_out=sc_half[:, t:t + 1],
                )
        # ---- constants ----
        identb = const.tile([128, 128], BF, tag="identb")
        nc.scalar.dma_start(out=identb[:], in_=identbd[:, :])
        ident32 = const.tile([128, 128], F32, tag="ident32")
        nc.scalar.dma_start(out=ident32[:], in_=ident32d[:, :])
        ones32 = const.tile([1, 16], F32, tag="ones32")
        nc.scalar.dma_start(out=ones32[:], in_=ones32d[:, :])
        iota16 = const.tile([16, 256], F32, tag="iota16")
        nc.scalar.dma_start(out=iota16[:], in_=iota16d[:, :])
        rep16 = const.tile([16, 128], F32, tag="rep16")
        nc.scalar.dma_start(out=rep16[:], in_=rep16d[:, :])
        onesb = const.tile([128, 8], BF, tag="onesb")
        nc.scalar.dma_start(out=onesb[:], in_=onesbd[:, :])
        trilq = const.tile([128, 128], BF, tag="trilq")
        nc.scalar.dma_start(out=trilq[:], in_=trilqd[:, :])

        # ---- two tiny pair AllGathers (the only collectives) ----
        ag_in = drp.tile([128, 16], F32, tag="agin")
        nc.scalar.dma_start(out=ag_in[:, :], in_=sc_half[:])
        ag_in2 = drp.tile([1, S // 2], F32, tag="agin2")
        nc.scalar.dma_start(out=ag_in2[:, :], in_=scrow[:])
        if collectives:
            nc.gpsimd.collective_compute(
                "AllGather", OP.bypass, replica_groups=groups,
                ins=[ag_in[:, :]], outs=[ag_out[:, :]],
            )
            nc.gpsimd.collective_compute(
                "AllGather", OP.bypass, replica_groups=groups,
                ins=[ag_in2[:, :]], outs=[ag_out2[:, :]],
            )
        else:
            nc.scalar.dma_start(out=ag_out[0:128, :], in_=ag_in[:, :])
            nc.scalar.dma_start(out=ag_out[128:256, :], in_=ag_in[:, :])
            nc.scalar.dma_start(out=ag_out2[0:1, :], in_=ag_in2[:, :])
            nc.scalar.dma_start(out=ag_out2[1:2, :], in_=ag_in2[:, :])
        nc.scalar.dma_start(out=scB[:, 0:16], in_=ag_out[0:128, :])
        nc.scalar.dma_start(out=scB[:, 16:32], in_=ag_out[128:256, :])
        scv = scores[:].rearrange("p (h t) -> p h t", h=2)
        for h in range(2):
            nc.scalar.dma_start(
                out=scv[:, h, :],
                in_=ag_out2[h:h + 1, :].rearrange("a (t p) -> p (a t)", p=128),
            )
        nc.vector.tensor_add(out=scores[:], in0=scores[:], in1=scB[:])

        # ---- phase 2: exact 512th-largest score + positions ----
        kv = const.tile([1, 2], F32, tag="kv")
        nc.gpsimd.kth_largest(out_ap=kv[:], in_ap=scores[:], n_per_lane=32,
                              k=510, quantile=1.0 - 510.5 / 4095.0)
        thr = ovr.tile([128, 16], F32, tag="ovr", name="thrps")
        nc.tensor.matmul(out=thr[:16, :1], lhsT=ones32[0:1, 0:16],
                         rhs=kv[0:1, 1:2], start=True, stop=True)

        # scores16[p, t*8+u] = scores[u*16+p, t] via 8 PE row-extract
        # matmuls (blocks at cols u*32) + one strided DVE copy.
        s16ps = mm.tile([128, 512], F32, tag="mm", name="s16ps")
        for u in range(8):
            nc.tensor.matmul(
                out=s16ps[0:16, u * 32:(u + 1) * 32],
                lhsT=ident32[:, u * 16:(u + 1) * 16], rhs=scores[:],
                start=True, stop=True,
            )
        scores16 = const.tile([16, 256], F32, tag="s16")
        s16v = scores16[:].rearrange("p (t u) -> p t u", u=8)
        nc.vector.tensor_copy(
            out=s16v[:, :, :],
            in_=s16ps[0:16, 0:256].rearrange("p (u t) -> p t u", u=8),
        )
        if debug:
            nc.scalar.dma_start(out=sc_dbg[:, :], in_=scores[:])
            nc.scalar.dma_start(out=s16_dbg[:, :], in_=scores16[:])
        m16 = const.tile([16, 256], F32, tag="m16")
        nc.vector.tensor_scalar(
            out=m16[:], in0=scores16[:], scalar1=thr[0:16, :1], scalar2=None,
            op0=OP.is_ge,
        )
        vals16 = const.tile([16, 256], F32, tag="v16")
        nc.vector.scalar_tensor_tensor(
            out=vals16[:], in0=iota16[:], scalar=1.0, in1=m16[:],
            op0=OP.add, op1=OP.mult,
        )
        nc.vector.tensor_scalar_add(vals16[:], vals16[:], -1.0)
        pos16f = const.tile([16, 32], F32, tag="p16f")
        nf_sb = const.tile([1, 1], U32, tag="nf")
        nc.gpsimd.sparse_gather(out=pos16f[:], in_=vals16[:],
                                num_found=nf_sb[:])
        pos16i = const.tile([16, 32], I32, tag="p16i")
        nc.vector.tensor_copy(out=pos16i[:], in_=pos16f[:])
        repps = ovr.tile([128, 16], F32, tag="ovr", name="repps")
        nc.tensor.matmul(out=repps[:, 0:16], lhsT=rep16[:],
                         rhs=pos16f[:, 0:16], start=True, stop=True)
        repps2 = ovr.tile([128, 16], F32, tag="ovr", name="repps2")
        nc.tensor.matmul(out=repps2[:, 0:16], lhsT=rep16[:],
                         rhs=pos16f[:, 16:32], start=True, stop=True)
        idx128 = const.tile([128, 32], I16, tag="idx128")
        nc.vector.tensor_copy(out=idx128[:, 0:16], in_=repps[:, 0:16])
        nc.vector.tensor_copy(out=idx128[:, 16:32], in_=repps2[:, 0:16])
        nc.scalar.dma_start(out=pos_out[:, :], in_=pos16i[:])
        nc.scalar.dma_start(out=nf_out[:, :], in_=nf_sb[:])

        # ---- phase 3: transposed gather -> tokT fp8 [128, 8, 512] ----
        # 16-bit-granularity transpose of fp8 rows: partition p, group g
        # holds the byte pair d = 2*(g*128+p)+j at free offset t*2+j;
        # the host's wqkv row permutation compensates.
        tokT = act.tile([128, 8, K], F8, tag="tokT")
        nc.gpsimd.dma_gather(
            out_ap=tokT[:, :, :], in_ap=xf8[:, :], idxs_ap=idx128[:, :],
            num_idxs=K, num_idxs_reg=K, elem_size=D, transpose=True,
        )
        if debug:
            nc.sync.dma_start(out=tokT_dbg[:, :, :], in_=tokT[:, :, :])
        tokTg = tokT[:].rearrange("p c t -> p (c t)").rearrange(
            "p (g t j) -> p g j t", g=4, j=2)
        # stationary (Ldweights) operands may not use the byte-interleaved
        # dual-fp8 layout: repack for the v matmul's lhsT
        tokT2 = act.tile([128, 4, 2, K], F8, tag="tokT2")
        for g in range(4):
            nc.vector.tensor_copy(out=tokT2[:, g, :, :], in_=tokTg[:, g, :, :])

        # ---- weights: the wire is FIFO, so gate the big loads with dummy
        # data deps; otherwise the scheduler hoists them ahead of the
        # latency-critical selection path. wqkv is gated on `scores` (it
        # fills the wire-idle selection window); the rest on tokT so the
        # gather is never queued behind them. ----
        wqkv_sb = wp.tile([128, 4, 2, 3 * D], F8, tag="wqkv")
        wproj_sb = wp.tile([128, 4, 2, D], F8, tag="wproj")
        wfc_sb = wp.tile([128, 4, 2, FCH], F8, tag="wfc")
        wout_sb = wp.tile([128, 8, 2, D], F8, tag="wout")
        dum = const.tile([1, 8], F32, tag="dum")
        nc.vector.tensor_copy(out=dum[0:1, 0:4], in_=scB[0:1, 0:4])
        nc.vector.tensor_copy(out=wqkv_sb[0:1, 0, 0, 0:16].bitcast(F32),
                              in_=dum[0:1, 0:4])
        nc.sync.dma_start(out=wqkv_sb[:, :, :, 0:2 * D],
                          in_=wqkv[:, :, :, 0:2 * D])
        nc.sync.dma_start(out=wqkv_sb[:, :, :, 2 * D:],
                          in_=wqkv[:, :, :, 2 * D:])
        dum2 = const.tile([1, 16], F8, tag="dum2")
        nc.vector.tensor_copy(out=dum2[0:1, 0:8], in_=tokT[0:1, 0, 0:8])
        nc.vector.tensor_copy(out=wproj_sb[0:1, 0, 0, 0:8],
                              in_=dum2[0:1, 0:8])
        nc.sync.dma_start(out=wproj_sb[:], in_=wproj[:, :, :, :])
        nc.vector.tensor_copy(out=wfc_sb[0:1, 0, 0, 0:8],
                              in_=dum2[0:1, 0:8])
        nc.sync.dma_start(out=wfc_sb[:], in_=wfc[:, :, :, :])
        nc.vector.tensor_copy(out=wout_sb[0:1, 0, 0, 0:8],
                              in_=dum2[0:1, 0:8])
        nc.sync.dma_start(out=wout_sb[:], in_=wout[:, :, :, :])

        # ---- phase 5: qkv (fp8 DoubleRow); q/k first, v after ----
        qT, kT = [], []
        for j in range(16):
            qk = mm.tile([128, 512], F32, tag="mm", name=f"qkps{j}")
            for g in range(4):
                nc.tensor.matmul(
                    out=qk[:], lhsT=wqkv_sb[:, g, :, j * 128:(j + 1) * 128],
                    rhs=tokTg[:, g, :, :],
                    start=(g == 0), stop=(g == 3), perf_mode=DR,
                )
            t = act.tile([128, K], BF, tag=f"qkT{j}", name=f"qkT{j}")
            if j < 8:
                nc.scalar.activation(out=t[:], in_=qk[:], func=AF.Copy,
                                     scale=0.125 * WSI)
                qT.append(t)
            else:
                if j % 2 == 0:
                    nc.vector.tensor_scalar_mul(t[:], qk[:], WSI)
                else:
                    nc.scalar.activation(out=t[:], in_=qk[:], func=AF.Copy,
                                         scale=WSI)
                kT.append(t)
        v_sb = act.tile([128, 4, D], BF, tag="v")
        for c in range(4):
            for n in range(2):
                vp = mm.tile([128, 512], F32, tag="mm", name=f"vps{c}_{n}")
                for g in range(4):
                    nc.tensor.matmul(
                        out=vp[:],
                        lhsT=tokT2[:, g, :, c * 128:(c + 1) * 128],
                        rhs=wqkv_sb[:, g, :, 2 * D + n * 512:2 * D + (n + 1) * 512],
                        start=(g == 0), stop=(g == 3), perf_mode=DR,
                    )
                if c % 2 == 0:
                    nc.vector.tensor_scalar_mul(
                        v_sb[:, c, n * 512:(n + 1) * 512], vp[:], WSI)
                else:
                    nc.scalar.activation(
                        out=v_sb[:, c, n * 512:(n + 1) * 512],
                        in_=vp[:], func=AF.Copy, scale=WSI)

        # ---- phase 6: attention (all 16 heads; transposed scores) ----
        oT_sb = act.tile([128, 4, 2, K], F8, tag="oT")
        of8_h = {}
        for h in range(H):
            jt, prt = h // 2, 64 * (h % 2)
            qTh = qT[jt][prt:prt + 64, :]
            kTh = kT[jt][prt:prt + 64, :]
            PT = ptp.tile([128, 4, K], BF, tag="PT", name=f"PT{h}")
            for kb in range(4):
                qn = K - kb * 128
                st = mm.tile([128, 512], F32, tag="mm", name=f"st{h}_{kb}")
                nc.tensor.matmul(
                    out=st[:, :qn], lhsT=kTh[:, kb * 128:(kb + 1) * 128],
                    rhs=qTh[:, kb * 128:], start=True, stop=True,
                )
                nc.scalar.activation(out=PT[:, kb, :qn], in_=st[:, :qn],
                                     func=AF.Exp)
                meng = nc.gpsimd if (h % 4 == 3) else nc.vector
                meng.tensor_mul(out=PT[:, kb, :128], in0=PT[:, kb, :128],
                                in1=trilq[:])
            of8 = sml.tile([128, 4, 64], BF, tag="of8", name=f"of8{h}")
            of8_h[h] = of8
            for qb in range(4):
                o_ps = ov.tile([128, 512], F32, tag="ov", name=f"ops{h}_{qb}")
                r_ps = ovr.tile([128, 16], F32, tag="ovr", name=f"rps{h}_{qb}")
                for c in range(qb + 1):
                    nc.tensor.matmul(
                        out=o_ps[:, 0:64],
                        lhsT=PT[:, c, (qb - c) * 128:(qb - c + 1) * 128],
                        rhs=v_sb[:, c, h * 64:(h + 1) * 64],
                        start=(c == 0), stop=(c == qb),
                    )
                    nc.tensor.matmul(
                        out=r_ps[:, 0:1],
                        lhsT=PT[:, c, (qb - c) * 128:(qb - c + 1) * 128],
                        rhs=onesb[:, 0:1],
                        start=(c == 0), stop=(c == qb),
                    )
                rc = sml.tile([128, 1], F32, tag="rc", name=f"rc{h}_{qb}")
                nc.vector.reciprocal(rc[:], r_ps[:, 0:1])
                nc.vector.tensor_scalar(
                    out=of8[:, qb, :], in0=o_ps[:, 0:64], scalar1=rc[:, :1],
                    scalar2=None, op0=OP.mult,
                )
            if h % 2 == 1:
                op_ps = trp.tile([128, 512], BF, tag="trp", name=f"otps{h}")
                for hh in (h - 1, h):
                    for qb in range(4):
                        nc.tensor.transpose(
                            out=op_ps[64 * (hh % 2):64 * (hh % 2) + 64,
                                      qb * 128:(qb + 1) * 128],
                            in_=of8_h[hh][:, qb, :],
                            identity=identb[:],
                        )
                p = h // 2
                nc.vector.tensor_copy(out=oT_sb[:, p // 2, p % 2, :],
                                      in_=op_ps[:])

        # ---- phase 7: proj (fp8 DoubleRow) -> attn rows bf16 ----
        attn_bf = act.tile([128, 4, D], BF, tag="attn")
        for tb in range(4):
            for n in range(2):
                pp = mm.tile([128, 512], F32, tag="mm", name=f"pj{tb}_{n}")
                for g in range(4):
                    nc.tensor.matmul(
                        out=pp[:], lhsT=oT_sb[:, g, :, tb * 128:(tb + 1) * 128],
                        rhs=wproj_sb[:, g, :, n * 512:(n + 1) * 512],
                        start=(g == 0), stop=(g == 3), perf_mode=DR,
                    )
                if n == 0:
                    nc.vector.tensor_scalar_mul(
                        attn_bf[:, tb, n * 512:(n + 1) * 512], pp[:], WSI)
                else:
                    nc.scalar.activation(
                        out=attn_bf[:, tb, n * 512:(n + 1) * 512],
                        in_=pp[:], func=AF.Copy, scale=WSI)
        if debug:
            nc.sync.dma_start(out=attn_dbg[:, :, :], in_=attn_bf[:, :, :])

        # ---- phase 8: layernorm -> xin bf16 (per-tb pipelined) ----
        xin = act.tile([128, 4, D], BF, tag="xin")
        for tb in range(4):
            at = attn_bf[:, tb, :]
            smt = sml.tile([128, 1], F32, tag="smt", name=f"smt{tb}")
            sqs = xsp.tile([128, 4, D], BF, tag="xr", name=f"sqs{tb}")
            ssq = sml.tile([128, 1], F32, tag="ssq", name=f"ssq{tb}")
            nc.scalar.activation(out=sqs[:, 0, :], in_=at, func=AF.Copy,
                                 accum_out=smt[:])
            nc.vector.scalar_tensor_tensor(
                out=sqs[:, 1, :], in0=at, scalar=0.0, in1=at,
                op0=OP.add, op1=OP.mult, accum_out=ssq[:],
            )
            mu = sml.tile([128, 1], F32, tag="mu", name=f"mu{tb}")
            nc.vector.tensor_scalar_mul(mu[:], smt[:], 1.0 / D)
            var = sml.tile([128, 1], F32, tag="var", name=f"var{tb}")
            nc.vector.tensor_scalar_mul(var[:], ssq[:], 1.0 / D)
            mu2 = sml.tile([128, 1], F32, tag="mu2", name=f"mu2{tb}")
            nc.vector.tensor_mul(out=mu2[:], in0=mu[:], in1=mu[:])
            nc.vector.tensor_sub(out=var[:], in0=var[:], in1=mu2[:])
            nc.vector.tensor_scalar_add(var[:], var[:], 1e-5)
            sd = sml.tile([128, 1], F32, tag="sd", name=f"sd{tb}")
            nc.scalar.activation(out=sd[:], in_=var[:], func=AF.Sqrt)
            rr = sml.tile([128, 1], F32, tag="rr", name=f"rr{tb}")
            nc.vector.reciprocal(rr[:], sd[:])
            xeng = nc.vector if tb % 2 == 0 else nc.gpsimd
            xeng.tensor_scalar(
                out=xin[:, tb, :], in0=at,
                scalar1=mu[:, :1], scalar2=rr[:, :1],
                op0=OP.subtract, op1=OP.mult,
            )

        # ---- phase 9: xiT fp8 [128, 4, 2, 512] ----
        xiT = act.tile([128, 4, 2, K], F8, tag="xiT")
        for g in range(4):
            for j in range(2):
                dc = g * 2 + j
                xps = trp.tile([128, 512], BF, tag="trp", name=f"xit{dc}")
                for tb in range(4):
                    nc.tensor.transpose(
                        out=xps[:, tb * 128:(tb + 1) * 128],
                        in_=xin[:, tb, dc * 128:(dc + 1) * 128],
                        identity=identb[:],
                    )
                nc.vector.tensor_copy(out=xiT[:, g, j, :], in_=xps[:])

        # ---- phase 10: fc + gelu (fp8 DoubleRow) ----
        hT = act.tile([128, 8, 2, K], F8, tag="hT")
        for f in range(16):
            fp = mm.tile([128, 512], F32, tag="mm", name=f"fc{f}")
            for g in range(4):
                nc.tensor.matmul(
                    out=fp[:], lhsT=wfc_sb[:, g, :, f * 128:(f + 1) * 128],
                    rhs=xiT[:, g, :, :],
                    start=(g == 0), stop=(g == 3), perf_mode=DR,
                )
            if not gelu_exact:
                hb = xsp.tile([128, 4, D], BF, tag="xr", name=f"hb{f}")
                nc.scalar.activation(out=hb[:, 0, 0:512], in_=fp[:],
                                     func=AF.Gelu_apprx_tanh, scale=WSI)
                heng = nc.vector if f % 2 == 0 else nc.gpsimd
                heng.tensor_copy(out=hT[:, f // 2, f % 2, :],
                                 in_=hb[:, 0, 0:512])
            else:
                # x*sigmoid(1.5957691*(x+0.044715*x^3)) via Exp+reciprocal
                hs = xsp.tile([128, 4, D], BF, tag="xr", name=f"gh{f}")
                x1 = hs[:, 0, 0:512]
                nc.vector.tensor_scalar_mul(x1, fp[:], WSI)
                h2 = hs[:, 1, 0:512]
                nc.vector.tensor_mul(out=h2, in0=x1, in1=x1)
                nc.vector.scalar_tensor_tensor(
                    out=h2, in0=h2, scalar=0.044715, in1=x1,
                    op0=OP.mult, op1=OP.mult,
                )
                nc.vector.tensor_add(out=h2, in0=h2, in1=x1)
                nc.scalar.activation(out=h2, in_=h2, func=AF.Exp,
                                     scale=-2.0 * 0.7978845608028654)
                nc.vector.tensor_scalar_add(h2, h2, 1.0)
                h3 = hs[:, 2, 0:512]
                with nc.allow_low_precision(reason="sigmoid denom, bf16 ok"):
                    nc.vector.reciprocal(h3, h2)
                nc.vector.tensor_mul(out=hT[:, f // 2, f % 2, :], in0=h3,
                                     in1=x1)

        # ---- phase 11: out partial (x64) -> updp bf16 ----
        upd_sb = act.tile([128, 4, D], BF, tag="updp")
        for tb in range(4):
            for n in range(2):
                op_ps = mm.tile([128, 512], F32, tag="mm", name=f"ou{tb}_{n}")
                for g in range(8):
                    nc.tensor.matmul(
                        out=op_ps[:], lhsT=hT[:, g, :, tb * 128:(tb + 1) * 128],
                        rhs=wout_sb[:, g, :, n * 512:(n + 1) * 512],
                        start=(g == 0), stop=(g == 7), perf_mode=DR,
                    )
                if n == 0:
                    nc.vector.tensor_copy(
                        out=upd_sb[:, tb, n * 512:(n + 1) * 512],
                        in_=op_ps[:])
                else:
                    nc.scalar.activation(
                        out=upd_sb[:, tb, n * 512:(n + 1) * 512],
                        in_=op_ps[:], func=AF.Copy)
            nc.sync.dma_start(out=updp[:, tb, :], in_=upd_sb[:, tb, :])

    nc.compile()
    return nc


_CACHE = {}


def _get_program(n_cores=8):
    if n_cores not in _CACHE:
        _CACHE[n_cores] = build_program(n_cores)
    return _CACHE[n_cores]


def _prep_shared(inputs):
    """Host-side weight shuffles/casts (shared by all cores)."""
    w_router = np.asarray(inputs["w_router"], np.float32)
    w_qkv = np.asarray(inputs["w_qkv"], np.float32)
    w_proj = np.asarray(inputs["w_proj"], np.float32)
    w_fc = np.asarray(inputs["w_fc"], np.float32)
    w_out = np.asarray(inputs["w_out"], np.float32)

    w64 = w_router[:, 0].astype(np.float32) * WS
    w8 = w64.astype(F8NP).astype(np.float32)
    ratio = np.divide(w8, w64, out=np.zeros_like(w64), where=w64 != 0)
    wrr = np.ascontiguousarray(
        np.broadcast_to(w64[None, :], (128, D))).astype(np.float32)
    # replicated-column fp8 router weights in the gather k-map layout:
    # w8rep[p, g, j, m] = w8[2*(g*128+p)+j]
    p_r = np.arange(128)
    dmapg = (2 * (np.arange(4)[None, :, None] * 128 + p_r[:, None, None])
             + np.arange(2)[None, None, :])
    w8rep = np.ascontiguousarray(np.broadcast_to(
        w8[dmapg][:, :, :, None], (128, 4, 2, 128)).astype(F8NP))
    gidx_h = []
    for e in range(2):
        g = np.zeros((16, 128), np.int16)
        for q in range(4):
            for j in range(32):
                for p in range(16):
                    g[p, q * 32 + j] = e * 2048 + q * 512 + j * 16 + p
        gidx_h.append(np.tile(g, (8, 1)))
    identb = np.eye(128, dtype=BF16NP)
    ident32 = np.eye(128, dtype=np.float32)
    iota16 = (np.arange(256)[None, :] * 16 + np.arange(16)[:, None]).astype(
        np.float32)
    rep16 = np.zeros((16, 128), np.float32)
    for p in range(128):
        rep16[p % 16, p] = 1.0
    onesb = np.ones((128, 8), BF16NP)
    ar = np.arange(128)
    trilq = (ar[None, :] >= ar[:, None]).astype(BF16NP)

    p_ = np.arange(128)
    g_ = np.arange(4)
    j_ = np.arange(2)
    # gather-layout row map: tokT partition p, group g, sub j holds
    # x row d = 2*(g*128+p)+j
    dmap_gather = (2 * (g_[None, :, None] * 128 + p_[:, None, None])
                   + j_[None, None, :])
    wqkv_f8 = np.ascontiguousarray(
        (w_qkv[dmap_gather.reshape(-1), :] * WS).reshape(128, 4, 2, 3 * D)
        .astype(F8NP))
    # standard DoubleRow k map: k row g*256 + j*128 + p
    dmap_std = (g_[None, :, None] * 256 + j_[None, None, :] * 128
                + p_[:, None, None])
    wproj_f8 = np.ascontiguousarray(
        (w_proj[dmap_std.reshape(-1), :] * WS).reshape(128, 4, 2, D)
        .astype(F8NP))
    halves = []
    g8 = np.arange(8)
    dmap8 = (g8[None, :, None] * 256 + j_[None, None, :] * 128
             + p_[:, None, None])
    for e in range(2):
        wfc_h = (w_fc[:, e * FCH:(e + 1) * FCH] * WS)
        wfc_f8 = np.ascontiguousarray(
            wfc_h[dmap_std.reshape(-1), :].reshape(128, 4, 2, FCH)
            .astype(F8NP))
        wout_h = (w_out[e * FCH:(e + 1) * FCH, :] * WS)
        wout_f8 = np.ascontiguousarray(
            wout_h[dmap8.reshape(-1), :].reshape(128, 8, 2, D).astype(F8NP))
        halves.append((wfc_f8, wout_f8))

    return dict(wrr=wrr, w8rep=w8rep, ratio=ratio, gidx_h=gidx_h,
                identb=identb, ident32=ident32, iota16=iota16,
                rep16=rep16, onesb=onesb, trilq=trilq,
                ones32=np.ones((1, 16), np.float32),
                wqkv_f8=wqkv_f8, wproj_f8=wproj_f8, halves=halves)


def make_in_maps(inputs, n_cores=8):
    x = np.asarray(inputs["x"], np.float32)
    sh = _prep_shared(inputs)
    xf8_all, xrp_all = [], []
    for b in range(B):
        xf8 = x[b].astype(F8NP)
        xrp = (x[b] - xf8.astype(np.float32) * sh["ratio"][None, :]
               ).astype(BF16NP)
        xf8_all.append(np.ascontiguousarray(xf8))
        xrp_all.append(xrp)

    in_maps = []
    for c in range(n_cores):
        b, e = (c // 2) % B, c % 2
        wfc_f8, wout_f8 = sh["halves"][e]
        in_maps.append({
            "xf8": xf8_all[b],
            "xrp": np.ascontiguousarray(
                xrp_all[b][e * (S // 2):(e + 1) * (S // 2)]),
            "gidx": sh["gidx_h"][e],
            "w8rep": sh["w8rep"],
            "wqkv": sh["wqkv_f8"],
            "wproj": sh["wproj_f8"],
            "wfc": wfc_f8,
            "wout": wout_f8,
            "wrouter_rep": sh["wrr"],
            "identb": sh["identb"],
            "ident32": sh["ident32"],
            "ones32": sh["ones32"],
            "iota16": sh["iota16"],
            "rep16": sh["rep16"],
            "onesb": sh["onesb"],
            "trilq": sh["trilq"],
        })
    return in_maps


def assemble_output(x, results):
    out = np.array(x, np.float32, copy=True)
    nb = len(results) // 2
    for b in range(nb):
        r0, r1 = results[2 * b], results[2 * b + 1]
        for r in (r0, r1):
            nf = int(np.asarray(r["nf_out"]).reshape(-1)[0])
            assert nf == K, f"batch {b}: expected {K} selected, got {nf}"
        pos = np.asarray(r0["pos_out"]).T.reshape(-1)     # [512] slot order
        u0 = np.asarray(r0["updp"]).astype(np.float32)    # [128, 4, 1024]
        u1 = np.asarray(r1["updp"]).astype(np.float32)
        part = (u0 + u1) * WSI
        rows = part.transpose(1, 0, 2).reshape(K, D)      # row s = tb*128+p
        out[b, pos] += rows
    return out


def kernel(**inputs):
    nc = _get_program(8)
    in_maps = make_in_maps(inputs, 8)
    res = run_bass_kernel_spmd(nc, in_maps, list(range(8))).results
    x = np.asarray(inputs["x"], np.float32)
    return assemble_output(x, res)


if __name__ == "__main__":
    nc = build_program(8)
    print("program built + compiled OK")

